# revision 91
# baseline (speedup 1.0000x reference)
"""Multi-head self-attention (B=2, T=2048, E=1024, H=16) on 8 TRN2 NeuronCores.

Sharding: tensor-parallel over heads (2 heads per core) for QKV + attention,
then four [8,64,256] AllToAlls (one per (head, batch)) reshard to
token-parallel for the output projection. Final gather/transpose on host.

Cost-model timeline 217.4us -> 194.1us (rel err 1.45e-2 vs 2e-2 budget).
Schedule notes (the exp stream on ACT ~133us and in-stream PE ~137us are
nearly balanced; the tail after the last exp is the other 35us):
  - QC=512 attention steps in (batch, head, q-chunk) order; scores matmuls
    feed exp through a 2-deep [128,2,512] PSUM pipe. Each step's av+norm is
    WOVEN into a later step's exp stream as 3 sub-pieces (hi/lo/Z+norm) --
    a block av would drain the pss pipe and stall ACT ~0.7us per step.
  - attn@V runs as fp8e4 DoubleRow matmuls (2x throughput only at M<=64 and
    contraction >=256: QKV/scores/out-proj gain nothing); V is split into
    e4m3 hi + lo residual so only exp quantization costs accuracy. Dropping
    the lo pass or carrying y/Q/K in fp8 all measured over the 2e-2 budget.
  - collectives cost a fixed 15us + bytes/40GBps and serialize on one
    resource: avs of steps 10-11 are pulled early (double-woven at si=12/13)
    so a2a(0,1) clears COLLECTIVE_CORES ~10us before a2a(1,1) is requested
    right after step 15's staging (~161.5us).
  - out-proj weights are split by head-slot (ow0/ow1, host-prearranged as
    [2 src cores x 64d, 4 core-pair tiles]): during the final collective PE
    runs every batch-1 piece's h0 half into auxps and parks the biased
    partial in SBUF (via idle ACT); after the collective only 4 matmuls +
    one DVE scalar_tensor_tensor + DMA per piece remain (~8us tail).
  - tc.tile_wait_until (scheduler-only hint, ignored by the final timeline)
    keeps epilogue work out of the in-order PE queue until its collective
    inputs exist -- otherwise the Tile scheduler hoists a piece whose ytf
    wait then head-of-line-blocks the exp stream ~6us.
  - ~17us of throwaway matmuls keep PE busy across the final collective:
    a long idle resets the cost model's p-state ramp and the tail matmuls
    would run at 0.65GHz instead of 2.4GHz (~5us slower).
  - every DMA is one merged HWDGE instruction (fixed 625ns per dma_start);
    weights arrive host-prearranged to their SBUF layouts; wq/x0 halves
    interleave so the first matmul starts ~3.3us in; the tail ytf and out
    DMAs are split across the sync/gpsimd queues to parallelize desc-gen.
"""

import numpy as np
import ml_dtypes

import concourse.bass as bass
import concourse.mybir as mybir
import concourse.tile as tile
from concourse import bacc, bass_utils

B, T, E, H, D = 2, 2048, 1024, 16, 64
NCORES = 8
HPC = H // NCORES            # heads per core = 2
FPC = HPC * D                # features per core = 128
TOK = B * T                  # 4096 global tokens
QC = 512                     # q chunk width (one attention step)
NQ2 = T // QC                # q chunks per batch = 4
NKT = T // 128               # k tiles per batch = 16
NIF = E // 128               # input-feature tiles = 8
XC = 512                     # token chunk width in QKV phase
NCH = TOK // XC              # QKV chunks = 8
TPB = T // NCORES            # tokens per dst core per batch = 256
NTT = TOK // 128             # token tiles for V' = 32
SCALE = 1.0 / float(np.sqrt(D))

F32 = mybir.dt.float32
BF16 = mybir.dt.bfloat16
FP8 = mybir.dt.float8e4
_BF = ml_dtypes.bfloat16

# 2 = V in fp8 hi + lo residual (exact V); 1 = hi only (saves ~13.7us of PE
# per core, but measured rel err 2.16e-2 exceeds the 2e-2 budget)
AV_PASSES = 2
# scale carried by y through the collectives (the out-proj bias activation
# divides it back out). With the bf16 payload there is nothing to gain from
# scaling; fp8e4 payload (YSC=32) halved the a2a cost but measured rel err
# 2.69e-2 -- over the 2e-2 budget.
YSC = 1.0


def build_nc():
    nc = bacc.Bacc(None, target_bir_lowering=False, num_devices=NCORES)

    xT = nc.declare_dram_parameter("xT", [E, TOK], BF16, isOutput=False)
    # weights arrive pre-arranged on host to the SBUF layout (contiguous
    # 2KB/partition rows -> cheap HWDGE descriptors)
    wq_p = nc.declare_dram_parameter("wq_p", [128, NIF * FPC], BF16, isOutput=False)
    wk_p = nc.declare_dram_parameter("wk_p", [128, NIF * FPC], BF16, isOutput=False)
    wv_p = nc.declare_dram_parameter("wv_p", [128, NIF * FPC], BF16, isOutput=False)
    bq = nc.declare_dram_parameter("bq", [FPC, 1], F32, isOutput=False)
    bk = nc.declare_dram_parameter("bk", [FPC, 1], F32, isOutput=False)
    bv = nc.declare_dram_parameter("bv", [FPC, 1], F32, isOutput=False)
    # out-proj weights split by head-slot: ow{h}_p[p, i, o, f] packs the
    # contraction features of head-slot h (64 rows per src core) as
    # [128 part = 2 src cores x 64 d, 4 core-pair tiles] so each half of a
    # piece is 4 K=128 matmuls -- the h0 half can run during the last
    # collective with the h1 half finishing the PSUM accumulation after it
    ow0_p = nc.declare_dram_parameter("ow0_p", [128, 4 * NIF * 128], BF16, isOutput=False)
    ow1_p = nc.declare_dram_parameter("ow1_p", [128, 4 * NIF * 128], BF16, isOutput=False)
    ob = nc.declare_dram_parameter("ob", [128, NIF], F32, isOutput=False)
    # columns: [b0 tokens c*256.. | b1 tokens c*256..]
    out = nc.declare_dram_parameter("out", [E, B * TPB], F32, isOutput=True)

    # per-(head, batch) collective bounce buffers
    cc_in = [
        [nc.dram_tensor(f"cc_in{h}_{b}", [NCORES, D, TPB], BF16) for b in range(B)]
        for h in range(HPC)
    ]
    cc_out = [
        [nc.dram_tensor(f"cc_out{h}_{b}", [NCORES, D, TPB], BF16) for b in range(B)]
        for h in range(HPC)
    ]

    with tile.TileContext(nc) as tc:
        with (
            tc.tile_pool(name="const", bufs=1) as cpool,
            tc.tile_pool(name="xt", bufs=5) as xpool,
            tc.tile_pool(name="aux", bufs=2, space="PSUM") as auxps,
            tc.tile_pool(name="pss", bufs=2, space="PSUM") as psspool,
            tc.tile_pool(name="psav", bufs=1, space="PSUM") as avpool,
            tc.tile_pool(name="psz", bufs=1, space="PSUM") as zpool,
            tc.tile_pool(name="exp", bufs=4) as epool,
            tc.tile_pool(name="small", bufs=4) as spool,
            tc.tile_pool(name="outp", bufs=16) as opool,
        ):
            # ---- persistent SBUF tensors ------------------------------
            wq_sb = cpool.tile([128, NIF, FPC], BF16, tag="wq")
            wk_sb = cpool.tile([128, NIF, FPC], BF16, tag="wk")
            wv_sb = cpool.tile([128, NIF, FPC], BF16, tag="wv")
            bq_sb = cpool.tile([FPC, 1], F32, tag="bq")
            bk_sb = cpool.tile([FPC, 1], F32, tag="bk")
            bv_sb = cpool.tile([FPC, 1], F32, tag="bv")
            # wq/x0 halves interleave on the sync queue so the first four Q
            # matmuls can start after ~2 DMAs instead of waiting for both
            # full tensors; biases are only needed at the DVE bias-adds, later

            qT_sb = cpool.tile([128, TOK], BF16, tag="qT")
            kT_sb = cpool.tile([128, TOK], BF16, tag="kT")
            # V in e4m3 hi + residual lo so the fp8 DoubleRow AV matmul only
            # pays the exp-quantization error, not V's. DoubleRow caps the
            # stationary free dim at 64, so no ones-column: the softmax
            # denominator comes from a separate M=1 ones matmul into row 64
            # of the same PSUM tile.
            vp_sb = cpool.tile([128, NTT, 2, D], FP8, tag="vp")
            vl_sb = cpool.tile([128, NTT, 2, D], FP8, tag="vl")
            # M=64 of ones: the Z matmul replicates the denominator into all
            # 64 rows of its PSUM tile (same N-bound cost as M=1), which also
            # gives the across-partitions broadcast of 1/Z for free
            ones2 = cpool.tile([128, 2, D], FP8, tag="ones2")
            nc.vector.memset(ones2, 1.0)

            yT_sb = cpool.tile([128, TOK], BF16, tag="yT")
            # ytf_sb[h][b]: gathered y for head-slot h, [128 part = 2 src
            # cores x 64 d, 4 core-pair tiles, TPB]
            ytf_sb = [
                [
                    cpool.tile([128, 4, TPB], BF16, tag=f"ytf{h}{b}", name=f"ytf{h}{b}")
                    for b in range(B)
                ]
                for h in range(HPC)
            ]

            ow_sb = [
                cpool.tile([128, 4, NIF, 128], BF16, tag=f"ow{h}", name=f"ow{h}")
                for h in range(HPC)
            ]
            ob_sb = cpool.tile([128, NIF], F32, tag="ob")

            # ---- QKV projection pieces for one 512-token chunk --------
            # one merged DMA per chunk: HWDGE charges a fixed 625ns per
            # dma_start, so batch the 8 feature tiles into one instruction
            xT_r = xT.rearrange("(i p) t -> p i t", p=128)

            def qkv_dma(c, split_dma=False):
                tsl = bass.ts(c, XC)
                xt = xpool.tile([128, NIF, XC], BF16, name="xt")
                if split_dma:
                    # first chunk: land the low feature tiles early so the
                    # first Q matmuls can start sooner
                    nc.sync.dma_start(out=xt[:, 0:4, :], in_=xT_r[:, 0:4, tsl])
                    nc.sync.dma_start(out=xt[:, 4:8, :], in_=xT_r[:, 4:8, tsl])
                else:
                    nc.sync.dma_start(out=xt, in_=xT_r[:, :, tsl])
                return xt

            qk_ps = {}

            def qkv_qk(c, xt, which, irange):
                """Half of a Q or K projection (4 of 8 accum matmuls)."""
                w_sb, b_sb, dst = (
                    (wq_sb, bq_sb, qT_sb) if which == "q" else (wk_sb, bk_sb, kT_sb)
                )
                key = (c, which)
                if key not in qk_ps:
                    qk_ps[key] = auxps.tile([128, XC], F32, tag="ps", name="qkps")
                ps = qk_ps[key]
                for i in irange:
                    nc.tensor.matmul(
                        ps,
                        lhsT=w_sb[:, i, :],
                        rhs=xt[:, i, :],
                        start=(i == 0),
                        stop=(i == NIF - 1),
                    )
                if irange[-1] == NIF - 1:
                    if which == "k" and c < 4:
                        # batch-0 K chunks: bias-add in halves so step 0's
                        # next two exp groups unlock ~0.5us sooner each
                        c0 = c * XC
                        nc.vector.tensor_scalar_add(
                            kT_sb[:, c0 : c0 + XC // 2], ps[:, 0 : XC // 2], b_sb
                        )
                        nc.vector.tensor_scalar_add(
                            kT_sb[:, c0 + XC // 2 : c0 + XC], ps[:, XC // 2 : XC], b_sb
                        )
                    else:
                        nc.vector.tensor_scalar_add(dst[:, bass.ts(c, XC)], ps, b_sb)
                    del qk_ps[key]

            def qkv_v(c, xt, tt4):
                psv = auxps.tile([128, FPC], F32, tag="ps")
                for i in range(NIF):
                    nc.tensor.matmul(
                        psv,
                        lhsT=xt[:, i, bass.ts(tt4, 128)],
                        rhs=wv_sb[:, i, :],
                        start=(i == 0),
                        stop=(i == NIF - 1),
                    )
                tt = c * (XC // 128) + tt4
                # V scaled by YSC at the split so y' = YSC*y flows through AV
                # and the collective in e4m3's normal range; the out-proj bias
                # activation divides YSC back out
                with nc.allow_low_precision(reason="V in split e4m3"):
                    for hh in range(2):
                        hi = vp_sb[:, tt, hh, :]
                        nc.vector.tensor_scalar_mul(
                            hi, psv[:, hh * D : (hh + 1) * D], float(YSC)
                        )
                        nc.vector.scalar_tensor_tensor(
                            vl_sb[:, tt, hh, :],
                            psv[:, hh * D : (hh + 1) * D],
                            float(YSC),
                            hi,
                            mybir.AluOpType.mult,
                            mybir.AluOpType.subtract,
                        )

            # ---- attention step pieces --------------------------------
            def scores_exp(step, ex=None, kt2_range=None, weave=None):
                """Scores+exp for a kt2 range; `weave` callables are emitted
                after each exp group to trickle filler PE work into the
                stream without starving the (2-deep) pss pipeline."""
                b, h, q2 = step
                hsl = slice(h * D, (h + 1) * D)
                qsl = bass.ds(b * T + q2 * QC, QC)
                if ex is None:
                    ex = epool.tile([128, NKT, QC], FP8, tag="ex")
                weave = list(weave or [])
                for kt2 in kt2_range or range(NKT // 2):
                    pss = psspool.tile([128, 2, QC], F32, tag="pss")
                    for j in range(2):
                        kt = 2 * kt2 + j
                        nc.tensor.matmul(
                            pss[:, j, :],
                            lhsT=kT_sb[hsl, bass.ds(b * T + kt * 128, 128)],
                            rhs=qT_sb[hsl, qsl],
                            start=True,
                            stop=True,
                        )
                    nc.scalar.activation(
                        out=ex[:, 2 * kt2 : 2 * kt2 + 2, :],
                        in_=pss,
                        func=mybir.ActivationFunctionType.Exp,
                        scale=float(SCALE),
                    )
                    if weave:
                        weave.pop(0)()
                for w in weave:
                    w()
                return ex

            def av_norm_pieces(step, ex, norm_eng=None):
                """The av+norm work of one step as 3 PE-sized pieces (hi pass,
                lo pass, Z+norm+staging) so it can be emitted as a block or
                woven between another step's exp groups."""
                b, h, q2 = step
                hsl = slice(h * D, (h + 1) * D)
                qsl = bass.ds(b * T + q2 * QC, QC)
                psav = avpool.tile([D, QC], F32, tag="av")
                psz = zpool.tile([D, QC], F32, tag="z")
                npair = NKT // 2

                # fp8 DoubleRow: each matmul contracts a pair of 128-token
                # tiles at half the per-row cost; hi then lo residual pass
                def av_pass(pi, v_sb):
                    for kp in range(npair):
                        tp = b * NKT + 2 * kp
                        nc.tensor.matmul(
                            psav,
                            lhsT=v_sb[:, tp : tp + 2, h, :],
                            rhs=ex[:, 2 * kp : 2 * kp + 2, :],
                            start=(pi == 0 and kp == 0),
                            stop=(pi == AV_PASSES - 1 and kp == npair - 1),
                            perf_mode=mybir.MatmulPerfMode.DoubleRow,
                        )

                def z_norm():
                    for kp in range(npair):
                        nc.tensor.matmul(
                            psz,
                            lhsT=ones2,
                            rhs=ex[:, 2 * kp : 2 * kp + 2, :],
                            start=(kp == 0),
                            stop=(kp == npair - 1),
                            perf_mode=mybir.MatmulPerfMode.DoubleRow,
                        )
                    # no V-bias here: softmax rows sum to 1, so bv's effect
                    # on the final output is the constant out_w @ bv, folded
                    # into the out-proj bias on the host
                    r_bc = spool.tile([D, QC], F32, tag="rbc")
                    ysl = yT_sb[hsl, qsl]
                    ne = norm_eng or nc.vector
                    with nc.allow_low_precision(reason="1/Z in f32/bf16"):
                        nc.vector.reciprocal(r_bc, psz)
                        ne.tensor_mul(ysl, psav, r_bc)
                    # stage this step's two dst-core chunks for the collective
                    # in ONE dma (DRAM side reordered to match the source)
                    nc.sync.dma_start(
                        out=cc_in[h][b][2 * q2 : 2 * q2 + 2].rearrange(
                            "g d t -> d g t"
                        ),
                        in_=yT_sb[hsl, qsl],
                    )

                pieces = [lambda: av_pass(0, vp_sb)]
                if AV_PASSES == 2:
                    pieces.append(lambda: av_pass(1, vl_sb))
                return pieces + [z_norm]

            def av_norm(step, ex, norm_eng=None):
                for piece in av_norm_pieces(step, ex, norm_eng):
                    piece()

            def scores_av_last(step):
                """Final step: AV/Z matmuls trail each exp group by one, so
                only one group's worth of PE work remains after the last exp
                and the final collective fires ~4us sooner."""
                b, h, q2 = step
                hsl = slice(h * D, (h + 1) * D)
                qsl = bass.ds(b * T + q2 * QC, QC)
                ex = epool.tile([128, NKT, QC], FP8, tag="ex", name="ex_last")
                psav = avpool.tile([D, QC], F32, tag="av", name="av_last")
                psz = zpool.tile([D, QC], F32, tag="z", name="z_last")
                npair = NKT // 2

                def av_group(kp):
                    tp = b * NKT + 2 * kp
                    for pi, v_sb in (((0, vp_sb), (1, vl_sb))[:AV_PASSES]):
                        nc.tensor.matmul(
                            psav,
                            lhsT=v_sb[:, tp : tp + 2, h, :],
                            rhs=ex[:, 2 * kp : 2 * kp + 2, :],
                            start=(pi == 0 and kp == 0),
                            stop=(pi == AV_PASSES - 1 and kp == npair - 1),
                            perf_mode=mybir.MatmulPerfMode.DoubleRow,
                        )
                    nc.tensor.matmul(
                        psz,
                        lhsT=ones2,
                        rhs=ex[:, 2 * kp : 2 * kp + 2, :],
                        start=(kp == 0),
                        stop=(kp == npair - 1),
                        perf_mode=mybir.MatmulPerfMode.DoubleRow,
                    )

                for kt2 in range(npair):
                    pss = psspool.tile([128, 2, QC], F32, tag="pss", name="pss_last")
                    for j in range(2):
                        kt = 2 * kt2 + j
                        nc.tensor.matmul(
                            pss[:, j, :],
                            lhsT=kT_sb[hsl, bass.ds(b * T + kt * 128, 128)],
                            rhs=qT_sb[hsl, qsl],
                            start=True,
                            stop=True,
                        )
                    nc.scalar.activation(
                        out=ex[:, 2 * kt2 : 2 * kt2 + 2, :],
                        in_=pss,
                        func=mybir.ActivationFunctionType.Exp,
                        scale=float(SCALE),
                    )
                    if kt2 >= 1:
                        av_group(kt2 - 1)
                av_group(npair - 1)
                # this step's staging gates the final collective: bf16 norm
                # chain (2x DVE; error impact of one step in 16 is nil)
                r_bc = spool.tile([D, QC], BF16, tag="rbc", name="rbc_last")
                ysl = yT_sb[hsl, qsl]
                with nc.allow_low_precision(reason="1/Z in bf16"):
                    nc.vector.reciprocal(r_bc, psz)
                nc.vector.tensor_mul(ysl, psav, r_bc)
                nc.sync.dma_start(
                    out=cc_in[h][b][2 * q2 : 2 * q2 + 2].rearrange("g d t -> d g t"),
                    in_=yT_sb[hsl, qsl],
                )

            def a2a(h, b):
                nc.gpsimd.collective_compute(
                    "AllToAll",
                    mybir.AluOpType.bypass,
                    replica_groups=[list(range(NCORES))],
                    ins=[cc_in[h][b][:].opt()],
                    outs=[cc_out[h][b][:].opt()],
                )

            def ytf_load(h, b):
                # gpsimd (Pool) HWDGE queue: its SEQ-held wait on the a2a
                # chains cleanly with the later collectives on the same queue
                nc.gpsimd.dma_start(
                    out=ytf_sb[h][b],
                    in_=cc_out[h][b].rearrange("(i two) d t -> (two d) i t", two=2),
                )

            def outproj_mms(b, o, ps, h, start, stop):
                for i in range(4):
                    nc.tensor.matmul(
                        ps,
                        lhsT=ow_sb[h][:, i, o, :],
                        rhs=ytf_sb[h][b][:, i, :],
                        start=(start and i == 0),
                        stop=(stop and i == 3),
                    )

            def outproj_bias_dma(b, o, pso, on_act, dma_eng=None):
                osb = opool.tile([128, TPB], F32, name="osb")
                if on_act:
                    # tail pieces: the exp stream is over, so the bias-add can
                    # ride the idle scalar engine instead of DVE; scale undoes
                    # the YSC carried by the fp8 y payload
                    nc.scalar.activation(
                        out=osb,
                        in_=pso,
                        func=mybir.ActivationFunctionType.Identity,
                        bias=ob_sb[:, o : o + 1],
                        scale=1.0 / YSC,
                    )
                elif YSC != 1.0:
                    nc.vector.tensor_scalar_mul(osb, pso, 1.0 / YSC)
                    nc.vector.tensor_scalar_add(osb, osb, ob_sb[:, o : o + 1])
                else:
                    nc.vector.tensor_scalar_add(osb, pso, ob_sb[:, o : o + 1])
                (dma_eng or nc.sync).dma_start(
                    out=out[o * 128 : (o + 1) * 128, bass.ds(b * TPB, TPB)], in_=osb
                )

            def outproj_piece(b, o, pool=None, on_act=False, dma_eng=None):
                pso = (pool or auxps).tile(
                    [128, TPB], F32, tag="ps" if pool is None else "pss", name="pso"
                )
                outproj_mms(b, o, pso, 0, True, False)
                outproj_mms(b, o, pso, 1, False, True)
                outproj_bias_dma(b, o, pso, on_act, dma_eng)

            # ---- emission schedule ------------------------------------
            # Weave the first two steps' score matmuls between the batch-0
            # QKV pieces so the exp stream on ACT saturates from ~5us on
            # (scores for k-tile j only need QKV chunk j//4), then trickle
            # the batch-1 QKV, the out-proj weights/pieces and the ytf loads
            # into the per-step PE slack under the ACT-bound softmax.
            steps = [(b, h, q2) for b in range(B) for h in range(HPC) for q2 in range(NQ2)]

            xts = {}
            wq_r = wq_p.rearrange("p (i f) -> p i f", i=NIF)
            wk_r = wk_p.rearrange("p (i f) -> p i f", i=NIF)
            xt0 = xpool.tile([128, NIF, XC], BF16, name="xt")
            nc.sync.dma_start(out=wq_sb[:, 0:4], in_=wq_r[:, 0:4])
            nc.sync.dma_start(out=xt0[:, 0:4, :], in_=xT_r[:, 0:4, bass.ts(0, XC)])
            nc.sync.dma_start(out=wq_sb[:, 4:8], in_=wq_r[:, 4:8])
            nc.sync.dma_start(out=xt0[:, 4:8, :], in_=xT_r[:, 4:8, bass.ts(0, XC)])
            xts[0] = xt0
            nc.sync.dma_start(out=wk_sb, in_=wk_r)
            # wv and the biases are not needed for several microseconds:
            # keep them off the HWDGE path so x chunk 1 lands before K1
            nc.gpsimd.dma_start(out=wv_sb, in_=wv_p.rearrange("p (i f) -> p i f", i=NIF))
            nc.gpsimd.dma_start(out=bq_sb, in_=bq[:, :])
            nc.gpsimd.dma_start(out=bk_sb, in_=bk[:, :])
            qkv_qk(0, xts[0], "q", range(0, 8))
            xts[1] = qkv_dma(1)
            qkv_qk(0, xts[0], "k", range(0, 8))
            ex1 = scores_exp(steps[0], kt2_range=range(0, 2))
            # K before Q per chunk: each K chunk immediately unlocks s1's
            # next two exp groups, Q is only needed a step later
            qkv_qk(1, xts[1], "k", range(0, 8))
            scores_exp(steps[0], ex=ex1, kt2_range=range(2, 4))
            qkv_qk(1, xts[1], "q", range(0, 8))
            xts[2] = qkv_dma(2)
            qkv_qk(2, xts[2], "k", range(0, 8))
            scores_exp(steps[0], ex=ex1, kt2_range=range(4, 6))
            qkv_qk(2, xts[2], "q", range(0, 8))
            xts[3] = qkv_dma(3)
            qkv_qk(3, xts[3], "k", range(0, 8))
            scores_exp(steps[0], ex=ex1, kt2_range=range(6, 8))
            qkv_qk(3, xts[3], "q", range(0, 8))
            # all of V(b0) must precede the first av (woven into step 3);
            # weave it into steps 1-2
            vkb0 = [
                (lambda cc, tt: lambda: qkv_v(cc, xts[cc], tt))(c, tt)
                for c in range(4)
                for tt in range(4)
            ]
            ex2 = scores_exp(steps[1], weave=vkb0[0:8])
            ex3 = scores_exp(steps[2], weave=vkb0[8:16])

            # out-proj weights: needed from the middle of phase 2 only; two
            # pieces so neither hogs the DMA engines for long
            nc.sync.dma_start(out=ob_sb, in_=ob[:, :])
            nc.sync.dma_start(
                out=ow_sb[0], in_=ow0_p.rearrange("p (i o f) -> p i o f", i=4, o=NIF)
            )
            nc.sync.dma_start(
                out=ow_sb[1], in_=ow1_p.rearrange("p (i o f) -> p i o f", i=4, o=NIF)
            )

            # per-step PE filler pieces (each ~0.4-0.9us), consumed by the
            # weave: batch-1 Q/K (needed by step si=8's scores), batch-1 V
            # (needed by si=11's av of the first b1 step), ytf loads and
            # out-proj b0 pieces once the b0 collectives are in
            def fills_for(si):
                w = []
                if si == 3:
                    w = [lambda: xts.__setitem__(4, qkv_dma(4)),
                         lambda: qkv_qk(4, xts[4], "q", range(0, 4)),
                         lambda: qkv_qk(4, xts[4], "q", range(4, 8)),
                         lambda: qkv_qk(4, xts[4], "k", range(0, 4))]
                elif si == 4:
                    w = [lambda: qkv_qk(4, xts[4], "k", range(4, 8)),
                         lambda: xts.__setitem__(5, qkv_dma(5)),
                         lambda: qkv_qk(5, xts[5], "q", range(0, 4)),
                         lambda: qkv_qk(5, xts[5], "q", range(4, 8))]
                elif si == 5:
                    w = [lambda: qkv_qk(5, xts[5], "k", range(0, 4)),
                         lambda: qkv_qk(5, xts[5], "k", range(4, 8)),
                         lambda: xts.__setitem__(6, qkv_dma(6)),
                         lambda: qkv_qk(6, xts[6], "q", range(0, 4))]
                elif si == 6:
                    w = [lambda: qkv_qk(6, xts[6], "q", range(4, 8)),
                         lambda: qkv_qk(6, xts[6], "k", range(0, 4)),
                         lambda: qkv_qk(6, xts[6], "k", range(4, 8)),
                         lambda: xts.__setitem__(7, qkv_dma(7))]
                elif si == 7:
                    w = [lambda: qkv_qk(7, xts[7], "q", range(0, 4)),
                         lambda: qkv_qk(7, xts[7], "q", range(4, 8)),
                         lambda: qkv_qk(7, xts[7], "k", range(0, 4)),
                         lambda: qkv_qk(7, xts[7], "k", range(4, 8))]
                elif si in (8, 9):
                    c = 4 + 2 * (si - 8)
                    w = [
                        (lambda cc, tt: lambda: qkv_v(cc, xts[cc], tt))(c + dc, tt)
                        for dc in range(2)
                        for tt in range(4)
                    ]
                return w

            # av schedule: default 3-deep pipeline (block av(si-3) before the
            # step's scores). avs of steps 11 and 12 are WOVEN into the exp
            # streams of si=12/13 (as 3 sub-pieces, offset 2 groups in) so
            # a2a(0,1) can fire ~25us before the end of the exp stream and
            # clear COLLECTIVE_CORES before a2a(1,1) is requested -- a block
            # double-av would drain the 2-deep pss pipe and stall ACT
            block_av = {15: 14}
            woven_av = {si: [si - 3] for si in range(3, 15)}
            woven_av[12] = [9, 11]
            woven_av[13] = [10, 12]
            woven_av[14] = [13]
            noop = lambda: None

            pending = {0: ex1, 1: ex2, 2: ex3}
            for si in range(3, len(steps)):
                if si in block_av:
                    av_norm(steps[block_av[si]], pending.pop(block_av[si]))
                weave = fills_for(si)

                for pi in woven_av.get(si, []):
                    weave = weave + [noop] * max(0, 2 - len(weave)) + av_norm_pieces(
                        steps[pi], pending.pop(pi)
                    )
                if si == 15:
                    # final step: av trails each exp group by one so only a
                    # sliver of PE work remains after the last exp
                    scores_av_last(steps[15])
                else:
                    pending[si] = scores_exp(steps[si], weave=weave)
                # fire the collective of any (h, b) quarter whose four avs
                # are all staged (the last one was woven into this step's exp
                # stream above), then queue the ytf load behind it on the
                # Pool HWDGE queue (SEQ-held waits chain in order there)
                if si == 6:
                    a2a(0, 0)
                elif si == 8:
                    ytf_load(0, 0)
                elif si == 10:
                    a2a(1, 0)
                    ytf_load(1, 0)
                elif si == 13:
                    # av(11) staged during si=12, av(10) woven just above
                    a2a(0, 1)
                    ytf_load(0, 1)


            # ---- epilogue ---------------------------------------------
            # a2a(1,1) fires as soon as step 15's staging DMA lands; PE rides
            # out the collective with the remaining b0 pieces and the h0
            # halves of every b1 piece (parked in PSUM); after the collective
            # only 4 matmuls + bias + DMA per b1 piece remain
            a2a(1, 1)
            # tile_wait_until only steers the SCHEDULER (bass_wait_until_ts
            # is ignored by the final cost timeline): keep each epilogue
            # phase out of the in-order PE queue until its inputs are there,
            # or a hoisted piece's not-yet-satisfied ytf wait parks at the
            # queue head and stalls the exp stream ~6us
            with tc.tile_wait_until(0.146):
                # out-DMAs on the gpsimd queue: these run exactly when step
                # 15's staging DMA (the final collective's gate) needs the
                # sync HWDGE -- queuing 625ns desc-gens ahead of it costs
                # ~2us on the critical tail
                for o in range(0, 6):
                    outproj_piece(0, o, on_act=True, dma_eng=nc.gpsimd)
            # stagger the remaining ready PE work across the final collective
            # window (~2us apart): short PE gaps keep the p-state ramp hot, so
            # the post-collective matmuls run at 2.4GHz instead of restarting
            # the ramp at 0.65GHz (~5us slower tail)
            osb0 = []
            for o in range(NIF):
                with tc.tile_wait_until(0.160 + 0.002 * o):
                    ps = auxps.tile([128, TPB], F32, tag="ps", name="p0")
                    outproj_mms(1, o, ps, 0, True, True)
                    t = opool.tile([128, TPB], F32, name="osb0")
                    # park h0 partial in SBUF with the bias and 1/YSC folded
                    # in; the h1 pass then just adds its own scaled partial
                    nc.scalar.activation(
                        out=t,
                        in_=ps,
                        func=mybir.ActivationFunctionType.Identity,
                        bias=ob_sb[:, o : o + 1],
                        scale=1.0 / YSC,
                    )
                    osb0.append(t)
            with tc.tile_wait_until(0.177):
                outproj_piece(0, 6, on_act=True, dma_eng=nc.gpsimd)
            with tc.tile_wait_until(0.180):
                outproj_piece(0, 7, on_act=True, dma_eng=nc.gpsimd)
            with tc.tile_wait_until(0.160):
                # split the tail-critical ytf across two HWDGE queues so the
                # two halves' descriptors generate and fly in parallel
                src11 = cc_out[1][1].rearrange("(i two) d t -> (two d) i t", two=2)
                nc.sync.dma_start(out=ytf_sb[1][1][:, 0:2], in_=src11[:, 0:2])
                nc.sync.dma_start(out=ytf_sb[1][1][:, 2:4], in_=src11[:, 2:4])
            with tc.tile_wait_until(0.168):
                # ~17us of throwaway matmuls keep PE busy across the final
                # collective: a >2us idle resets the p-state ramp and the
                # post-collective matmuls would run at 0.65GHz instead of
                # 2.4GHz (~5us slower tail). The filler ends just before the
                # ytf lands, so it delays nothing.
                fill_ps = psspool.tile([128, 2, QC], F32, tag="pss", name="fill")
                for _ in range(80):
                    nc.tensor.matmul(
                        fill_ps[:, 0, :],
                        lhsT=wq_sb[:, 0, :],
                        rhs=qT_sb[:, 0:QC],
                        start=True,
                        stop=True,
                    )
            with tc.tile_wait_until(0.183):
                # 4-deep PSUM rotation (aux/av/aux/z) so piece o+4's matmuls
                # never wait on piece o's DVE add to release the bank; output
                # DMAs batched per o-pair and spread over 4 HWDGE queues so
                # the tail isn't 8 serial 625ns descriptor generations
                tail_pools = [auxps, avpool, auxps, zpool]
                dma_engs = (nc.sync, nc.gpsimd, nc.sync, nc.gpsimd)
                for op in range(4):
                    osb = opool.tile([128, 2, TPB], F32, name="osb")
                    for j in range(2):
                        o = 2 * op + j
                        pool = tail_pools[o % 4]
                        tag = {id(auxps): "ps", id(avpool): "av", id(zpool): "z"}[
                            id(pool)
                        ]
                        ps = pool.tile([128, TPB], F32, tag=tag, name="p1")
                        outproj_mms(1, o, ps, 1, True, True)
                        nc.vector.scalar_tensor_tensor(
                            osb[:, j, :],
                            ps,
                            1.0 / YSC,
                            osb0[o],
                            mybir.AluOpType.mult,
                            mybir.AluOpType.add,
                        )
                    dma_engs[op].dma_start(
                        out=out[
                            (2 * op) * 128 : (2 * op + 2) * 128, bass.ds(TPB, TPB)
                        ].rearrange("(o p) t -> p o t", o=2),
                        in_=osb,
                    )

    nc.finalize()
    return nc


def make_in_maps(x, qkv_w, qkv_b, out_w, out_b):
    x = np.asarray(x, dtype=np.float32).reshape(TOK, E)
    qkv_w = np.asarray(qkv_w, dtype=np.float32)
    qkv_b = np.asarray(qkv_b, dtype=np.float32)
    out_w = np.asarray(out_w, dtype=np.float32)
    out_b = np.asarray(out_b, dtype=np.float32)

    xT = np.ascontiguousarray(x.T).astype(_BF)
    owT = np.ascontiguousarray(out_w.T)
    # SBUF layouts prepared host-side: w_p[p, i*FPC+f] = wT[i*128+p, f]
    def prearrange_w(wT):
        return np.ascontiguousarray(
            wT.reshape(NIF, 128, FPC).transpose(1, 0, 2).reshape(128, NIF * FPC)
        ).astype(_BF)

    # h-split out-proj: ow{h}_p[p, i, o*128+f] = owT[(2i + p//64)*128 +
    # h*64 + (p%64), o*128+f] -- contraction rows for head-slot h packed
    # as [2 src cores x 64 d, 4 core-pair tiles]
    p_idx = np.arange(128)
    i_idx = np.arange(4)
    feat = (2 * i_idx[None, :] + p_idx[:, None] // 64) * 128 + (p_idx[:, None] % 64)
    ow0_p = np.ascontiguousarray(owT[feat].reshape(128, 4 * E)).astype(_BF)
    ow1_p = np.ascontiguousarray(owT[feat + 64].reshape(128, 4 * E)).astype(_BF)
    # V-bias folded here: softmax weights sum to 1, so every token's y gets
    # exactly +bv and the output gets the constant out_w @ bv
    ob_eff = out_b + out_w @ qkv_b[2 * E : 3 * E]
    ob = np.ascontiguousarray(ob_eff.reshape(NIF, 128).T.astype(np.float32))

    in_maps = []
    for c in range(NCORES):
        rs = slice(c * FPC, (c + 1) * FPC)
        ks = slice(E + c * FPC, E + (c + 1) * FPC)
        vs = slice(2 * E + c * FPC, 2 * E + (c + 1) * FPC)
        in_maps.append(
            {
                "xT": xT,
                "wq_p": prearrange_w(qkv_w[rs, :].T),
                "wk_p": prearrange_w(qkv_w[ks, :].T),
                "wv_p": prearrange_w(qkv_w[vs, :].T),
                "bq": qkv_b[rs].reshape(FPC, 1).copy(),
                "bk": qkv_b[ks].reshape(FPC, 1).copy(),
                "bv": (qkv_b[vs] * YSC).reshape(FPC, 1).copy(),
                "ow0_p": ow0_p,
                "ow1_p": ow1_p,
                "ob": ob,
            }
        )
    return in_maps


def assemble(results):
    full = np.empty((TOK, E), dtype=np.float32)
    for c in range(NCORES):
        o = results[c]["out"]
        full[c * TPB : (c + 1) * TPB, :] = o[:, 0:TPB].T
        full[T + c * TPB : T + (c + 1) * TPB, :] = o[:, TPB : 2 * TPB].T
    return full.reshape(B, T, E)


_NC_CACHE = None


def kernel(x, qkv_w, qkv_b, out_w, out_b):
    global _NC_CACHE
    if _NC_CACHE is None:
        _NC_CACHE = build_nc()
    in_maps = make_in_maps(x, qkv_w, qkv_b, out_w, out_b)
    res = bass_utils.run_bass_kernel_spmd(
        _NC_CACHE, in_maps, core_ids=list(range(NCORES))
    )
    return assemble(res.results)

